# revision 48
# baseline (speedup 1.0000x reference)
"""Mamba2 mixer kernel for 8 trn2 NeuronCores, tensor-parallel over heads.

Core k handles heads 8k..8k+7 (d_inner channels 512k..512k+512):
  - in_proj slices in bf16 (z, x, dt columns; B/C redundant on all cores)
  - causal depthwise conv on PE (diagonal matmuls), silu in tanh form
  - chunked SSD scan (chunk 128) for its 8 heads, bf16 matmuls,
    software-pipelined one block behind in_proj so PE never waits on the
    conv -> silu -> transpose chain
  - dt softplus via exp/ln; cumsum via lower-tri matmul
  - decay-row broadcast via DMA bounce through DRAM
  - gated output y*silu(z) kept in SBUF (bf16); RMSNorm partial sums of
    squares AllGathered in 4 pipelined groups and reduced locally.
Host converts inputs to bf16 / builds small constant matrices, and
concatenates the 8 per-core channel-slice outputs.
"""

import numpy as np
import ml_dtypes

import concourse.bass as bass
import concourse.tile as tile
from concourse import bacc, mybir
from concourse.bass_utils import run_bass_kernel_spmd

F32 = mybir.dt.float32
BF16 = mybir.dt.bfloat16
AF = mybir.ActivationFunctionType
ALU = mybir.AluOpType
BF = ml_dtypes.bfloat16

# dims
B_, L_, DM = 2, 2048, 2048
DS, DC, HD = 128, 4, 64
DI = 2 * DM              # 4096
NCORE = 8
HPC = 8                  # heads per core
XC = 512                 # x/z channels per core
T = B_ * L_              # 4096 tokens
BLK = 512                # tokens per main-loop block
NBLK = T // BLK          # 8
CHK = 128                # SSD chunk
NCH = T // CHK           # 32
NKB = DM // 128          # 16 contraction blocks
EPS = 1e-5
AG_SIZES = (16, 8, 6, 2)  # collective group sizes (chunks)
AG_OFF = (0, 16, 24, 30)


def _build_nc():
    nc = bacc.Bacc("TRN2", target_bir_lowering=False, num_devices=NCORE)

    uT = nc.dram_tensor("uT", [DM, T], BF16, kind="ExternalInput")
    wxbc = nc.dram_tensor("wxbc", [DM, 768], BF16, kind="ExternalInput")
    wz = nc.dram_tensor("wz", [DM, XC], BF16, kind="ExternalInput")
    wdt = nc.dram_tensor("wdt", [DM, HPC], BF16, kind="ExternalInput")
    convd = nc.dram_tensor("convd", [128, 24 * 128], BF16, kind="ExternalInput")
    cvec = nc.dram_tensor("cvec", [128, 6], F32, kind="ExternalInput")
    hvec = nc.dram_tensor("hvec", [128, 6 * 3], F32, kind="ExternalInput")
    dtbb = nc.dram_tensor("dtbb", [128, 4 * HPC], F32, kind="ExternalInput")
    abc = nc.dram_tensor("abc", [128, 4 * HPC], F32, kind="ExternalInput")
    dids = nc.dram_tensor("dids", [128, HPC * 128], BF16, kind="ExternalInput")
    nrmb = nc.dram_tensor("nrmb", [128, XC], BF16, kind="ExternalInput")
    trim = nc.dram_tensor("trim", [128, 128], F32, kind="ExternalInput")
    maskm = nc.dram_tensor("maskm", [128, CHK], F32, kind="ExternalInput")
    idf = nc.dram_tensor("idf", [128, 128], F32, kind="ExternalInput")

    out = nc.dram_tensor("out", [T, XC], BF16, kind="ExternalOutput")

    uTr = uT.rearrange("(o p) t -> p o t", p=128)

    from contextlib import ExitStack

    with tile.TileContext(nc) as tc, ExitStack() as stack:
        ep = lambda *a, **k: stack.enter_context(tc.tile_pool(*a, **k))
        wpool = ep(name="wpool", bufs=1)
        cpool = ep(name="cpool", bufs=1)
        upool = ep(name="upool", bufs=2)
        xsp = ep(name="xsp", bufs=2)
        convp = ep(name="convp", bufs=2)
        zsp = ep(name="zsp", bufs=2)
        dtp = ep(name="dtp", bufs=3)
        trp = ep(name="trp", bufs=4)
        pabp = ep(name="pabp", bufs=3)
        estp = ep(name="estp", bufs=2)
        hw = ep(name="hw", bufs=3)
        hw2 = ep(name="hw2", bufs=2)
        rp = ep(name="rp", bufs=2)
        yp = ep(name="yp", bufs=2)
        ygp = ep(name="ygp", bufs=1)
        otp = ep(name="otp", bufs=2)
        ps_io = ep(name="ps_io", bufs=4, space="PSUM")
        ps_ssd = ep(name="ps_ssd", bufs=3, space="PSUM")
        ps_sm = ep(name="ps_sm", bufs=1, space="PSUM")
        dram = ep(name="dram", bufs=1, space="DRAM")
        if True:
            # ------------- weights / constants (startup-critical first) --
            ut_tiles = {}

            def prefetch_ut(p):
                utile = upool.tile([128, NKB, BLK], BF16, tag="ut", name=f"ut{p}")
                for q in range(4):
                    nc.sync.dma_start(
                        utile[:, 4 * q : 4 * q + 4, :],
                        uTr[:, 4 * q : 4 * q + 4, p * BLK : (p + 1) * BLK],
                    )
                ut_tiles[p] = utile

            prefetch_ut(0)
            wdt_sb = wpool.tile([128, NKB, HPC], BF16)
            nc.sync.dma_start(wdt_sb[:], wdt.rearrange("(o p) c -> p o c", p=128))
            dtb_sb = cpool.tile([128, 4, HPC], F32)
            nc.sync.dma_start(dtb_sb[:], dtbb[:].rearrange("p (a b) -> p a b", a=4))
            a_sb = cpool.tile([128, 4, HPC], F32)
            nc.sync.dma_start(a_sb[:], abc[:].rearrange("p (a b) -> p a b", a=4))
            tri_sb = cpool.tile([128, 128], F32)
            nc.sync.dma_start(tri_sb[:], trim[:])
            id_sb = cpool.tile([128, 128], F32)
            nc.sync.dma_start(id_sb[:], idf[:])
            wx_sb = wpool.tile([128, NKB, 768], BF16)
            for q in range(2):
                nc.sync.dma_start(
                    wx_sb[:, 8 * q : 8 * q + 8, :],
                    wxbc.rearrange("(o p) c -> p o c", p=128)[:, 8 * q : 8 * q + 8, :],
                )
            wz_sb = wpool.tile([128, NKB, XC], BF16)
            nc.sync.dma_start(wz_sb[:], wz.rearrange("(o p) c -> p o c", p=128))
            cwd_sb = cpool.tile([128, 24, 128], BF16)
            nc.sync.dma_start(cwd_sb[:], convd[:].rearrange("p (a b) -> p a b", b=128))
            cv_sb = cpool.tile([128, 6], F32)
            nc.sync.dma_start(cv_sb[:], cvec[:])
            hv_sb = cpool.tile([128, 6, 3], F32)
            nc.sync.dma_start(hv_sb[:], hvec[:].rearrange("p (a b) -> p a b", b=3))
            did_sb = cpool.tile([128, HPC, 128], BF16)
            nc.sync.dma_start(did_sb[:], dids[:].rearrange("p (h t) -> p h t", h=HPC))
            nrm_sb = cpool.tile([128, XC], BF16)
            nc.sync.dma_start(nrm_sb[:], nrmb[:])
            mk_sb = cpool.tile([128, CHK], F32)
            nc.sync.dma_start(mk_sb[:], maskm[:])

            partial = cpool.tile([128, NCH], F32)

            acs_d = dram.tile([NBLK, HPC, 4, CHK], F32)
            cc_in = [dram.tile([128, n], F32, name=f"ccin{i}")
                     for i, n in enumerate(AG_SIZES)]
            cc_out = [dram.tile([NCORE, 128, n], F32, name=f"ccout{i}")
                      for i, n in enumerate(AG_SIZES)]

            # persistent state
            state = dict(R=None, yg=None, xsrc=None)
            dsp_all = {}
            acs_all = {}
            conv_out = {}   # block -> (xc, bcv, zs)
            tr_tiles = {}   # global chunk -> (BT, xT)
            pab_tiles = {}

            def dt_chain(p):
                utile = ut_tiles[p]
                dd = dtp.tile([128, 4, HPC], F32, tag="dd", name=f"dd{p}")
                for tb in range(4):
                    psm = ps_sm.tile([128, 128], F32, tag="sm", name=f"pdt{p}_{tb}")
                    pdt = psm[:, 0:HPC]
                    for kb in range(NKB):
                        nc.tensor.matmul(
                            pdt,
                            utile[:, kb, tb * 128 : (tb + 1) * 128],
                            wdt_sb[:, kb, :],
                            start=(kb == 0),
                            stop=(kb == NKB - 1),
                        )
                    nc.vector.tensor_tensor(
                        dd[:, tb, :], pdt, dtb_sb[:, tb, :], ALU.add
                    )
                # softplus = ln(1 + e^x), batched over the 4 token tiles
                dsp = dtp.tile([128, 4, HPC], F32, tag="dsp", name=f"dsp{p}")
                nc.scalar.activation(
                    dsp[:].rearrange("p a b -> p (a b)"),
                    dd[:].rearrange("p a b -> p (a b)"),
                    AF.Exp,
                )
                nc.scalar.activation(
                    dsp[:].rearrange("p a b -> p (a b)"),
                    dsp[:].rearrange("p a b -> p (a b)"),
                    AF.Ln,
                    bias=1.0,
                )
                dA = dtp.tile([128, 4, HPC], F32, tag="dA", name=f"dA{p}")
                nc.vector.tensor_tensor(
                    dA[:].rearrange("p a b -> p (a b)"),
                    dsp[:].rearrange("p a b -> p (a b)"),
                    a_sb[:].rearrange("p a b -> p (a b)"),
                    ALU.mult,
                )
                acs = dtp.tile([128, 4, HPC], F32, tag="acs", name=f"acs{p}")
                aht = dtp.tile([HPC, 4, CHK], F32, tag="aht", name=f"aht{p}")
                for cc in range(4):
                    psa = ps_sm.tile([128, 128], F32, tag="sm", name=f"pa{p}_{cc}")
                    pacs = psa[:, 0:HPC]
                    nc.tensor.matmul(
                        pacs, tri_sb[:], dA[:, cc, :], start=True, stop=True
                    )
                    nc.vector.tensor_copy(acs[:, cc, :], pacs)
                    psh = ps_sm.tile([128, 128], F32, tag="sm", name=f"ph{p}_{cc}")
                    paht = psh[0:HPC, :]
                    nc.tensor.transpose(paht, acs[:, cc, :], id_sb[:])
                    nc.scalar.copy(aht[:, cc, :], paht)
                nc.sync.dma_start(acs_d[p], aht[:])
                dsp_all[p] = dsp
                acs_all[p] = acs

            def issue_pab(g):
                p, cc = g // 4, g % 4
                pab = pabp.tile([128, HPC, CHK], F32, tag="pab", name=f"pab{g}")
                nc.sync.dma_start(
                    pab[:], acs_d[p][None, :, cc, :].to_broadcast((128, HPC, CHK))
                )
                pab_tiles[g] = pab

            def issue_transposes(p):
                """DMA-transpose B and x for all 4 chunks of block p."""
                xc, bcv, _ = conv_out[p]
                for cc in range(4):
                    g = p * 4 + cc
                    tsl = slice(cc * CHK, (cc + 1) * CHK)
                    BT = trp.tile([128, 128], BF16, tag="BT", name=f"BT{g}")
                    nc.sync.dma_start_transpose(BT[:], bcv[:, 0, tsl])
                    xT = trp.tile([128, XC], BF16, tag="xT", name=f"xT{g}")
                    for xp in range(4):
                        nc.sync.dma_start_transpose(
                            xT[:, xp * 128 : (xp + 1) * 128], xc[:, xp, tsl]
                        )
                    tr_tiles[g] = (BT, xT)

            def in_proj_block(p):
                utile = ut_tiles[p]
                xsrc = xsp.tile([128, 6, BLK + 3], BF16, tag="xsrc", name=f"xs{p}")
                for cb in range(6):
                    p1 = ps_io.tile([128, BLK], F32, tag="io", name=f"p1_{p}_{cb}")
                    for kb in range(NKB):
                        nc.tensor.matmul(
                            p1[:],
                            wx_sb[:, kb, cb * 128 : (cb + 1) * 128],
                            utile[:, kb, :],
                            start=(kb == 0),
                            stop=(kb == NKB - 1),
                        )
                    nc.scalar.activation(
                        xsrc[:, cb, 3 : BLK + 3], p1[:], AF.Identity,
                        bias=cv_sb[:, cb : cb + 1],
                    )
                if p % (NBLK // B_) == 0:
                    nc.vector.tensor_copy(xsrc[:, :, 0:3], hv_sb[:])
                else:
                    nc.vector.tensor_copy(
                        xsrc[:, :, 0:3], state["xsrc"][:, :, BLK : BLK + 3]
                    )
                state["xsrc"] = xsrc

                zs = zsp.tile([128, 4, XC], BF16, tag="zs", name=f"zs{p}")
                for tb in range(4):
                    pz = ps_io.tile([128, XC], F32, tag="io", name=f"pz{p}_{tb}")
                    for kb in range(NKB):
                        nc.tensor.matmul(
                            pz[:],
                            utile[:, kb, tb * 128 : (tb + 1) * 128],
                            wz_sb[:, kb, :],
                            start=(kb == 0),
                            stop=(kb == NKB - 1),
                        )
                    tz = hw2.tile([128, XC], BF16, tag="tz")
                    nc.scalar.activation(tz[:], pz[:], AF.Tanh)
                    nc.vector.scalar_tensor_tensor(
                        zs[:, tb, :], tz[:], 1.0, pz[:], ALU.add, ALU.mult
                    )

                if p + 1 < NBLK:
                    dt_chain(p + 1)

                xc = convp.tile([128, 4, BLK], BF16, tag="xc", name=f"xc{p}")
                bcv = convp.tile([128, 2, BLK], BF16, tag="bcv", name=f"bc{p}")
                for cb in range(6):
                    pcv = ps_io.tile([128, BLK], F32, tag="io", name=f"pc{p}_{cb}")
                    for k in range(4):
                        nc.tensor.matmul(
                            pcv[:],
                            cwd_sb[:, 4 * cb + k, :],
                            xsrc[:, cb, k : k + BLK],
                            start=(k == 0),
                            stop=(k == 3),
                        )
                    tcv = hw2.tile([128, BLK], BF16, tag="tcv")
                    nc.scalar.activation(tcv[:], pcv[:], AF.Tanh)
                    dst = xc[:, cb, :] if cb < 4 else bcv[:, cb - 4, :]
                    nc.vector.scalar_tensor_tensor(
                        dst, tcv[:], 1.0, pcv[:], ALU.add, ALU.mult
                    )
                conv_out[p] = (xc, bcv, zs)

            def ssd_block(p):
                xc, bcv, zs = conv_out.pop(p)
                dsp = dsp_all.pop(p)
                acs = acs_all.pop(p)
                for cc in range(4):
                    g = p * 4 + cc
                    first = g % (NCH // B_) == 0
                    tsl = slice(cc * CHK, (cc + 1) * CHK)
                    pab = pab_tiles.pop(g)
                    if g + 2 < NCH:
                        issue_pab(g + 2)
                    BT, xT = tr_tiles.pop(g)

                    w2 = hw2.tile([128, HPC, CHK], F32, tag="w2")
                    for h in range(HPC):
                        nc.vector.tensor_scalar(
                            w2[:, h, :],
                            pab[:, h, :],
                            acs[:, cc, h : h + 1],
                            0.0,
                            ALU.subtract,
                            ALU.min,
                        )
                    est8 = estp.tile([128, HPC, CHK], BF16, tag="est8")
                    for hh in range(2):
                        nc.scalar.activation(
                            est8[:, 4 * hh : 4 * hh + 4, :].rearrange(
                                "p h t -> p (h t)"
                            ),
                            w2[:, 4 * hh : 4 * hh + 4, :].rearrange(
                                "p h t -> p (h t)"
                            ),
                            AF.Exp,
                        )
                    el0 = hw.tile([128, HPC], F32, tag="el0")
                    nc.vector.tensor_tensor(
                        el0[:],
                        pab[:, :, CHK - 1 : CHK].rearrange("p h o -> p (h o)"),
                        acs[:, cc, :],
                        ALU.subtract,
                    )
                    eal = hw.tile([128, 2, HPC], F32, tag="eal", name=f"el{g}")
                    nc.scalar.activation(eal[:, 1, :], el0[:], AF.Exp)
                    nc.scalar.activation(
                        eal[:, 0, :],
                        pab[:, :, CHK - 1 : CHK].rearrange("p h o -> p (h o)"),
                        AF.Exp,
                    )
                    ea_b = estp.tile([128, HPC, CHK], BF16, tag="eab")
                    if not first:
                        nc.scalar.activation(
                            ea_b[:].rearrange("p h t -> p (h t)"),
                            pab[:].rearrange("p h t -> p (h t)"),
                            AF.Exp,
                        )

                    pbc = ps_sm.tile([128, 128], F32, tag="sm", name=f"pbc{g}")
                    nc.tensor.matmul(
                        pbc[:, 0:CHK], bcv[:, 0, tsl], bcv[:, 1, tsl],
                        start=True, stop=True,
                    )
                    bcm = hw.tile([128, CHK], BF16, tag="bcm", name=f"bm{g}")
                    nc.vector.tensor_tensor(bcm[:], pbc[:, 0:CHK], mk_sb[:], ALU.mult)

                    py = ps_ssd.tile([128, XC], F32, tag="ssd", name=f"py{g}")
                    Ce = None
                    if not first:
                        Ce = estp.tile([128, HPC, CHK], BF16, tag="Ce")
                        for h in range(HPC):
                            eng = nc.gpsimd if h < 4 else nc.vector
                            eng.tensor_tensor(
                                Ce[:, h, :], bcv[:, 1, tsl], ea_b[:, h, :],
                                ALU.mult,
                            )
                    xs = estp.tile([128, XC], BF16, tag="xs", name=f"xv{g}")
                    for h in range(HPC):
                        hsl = slice(h * HD, (h + 1) * HD)
                        eng = nc.vector if h < 4 else nc.gpsimd
                        eng.tensor_scalar(
                            xs[:, hsl],
                            xT[:, hsl],
                            eal[:, 1, h : h + 1],
                            dsp[:, cc, h : h + 1],
                            ALU.mult,
                            ALU.mult,
                        )
                    pst = ps_ssd.tile([128, XC], F32, tag="ssd", name=f"pst{g}")
                    nc.tensor.matmul(pst[:], BT[:], xs[:], start=True, stop=True)
                    for h in range(HPC):
                        hsl = slice(h * HD, (h + 1) * HD)
                        M = hw.tile([128, CHK], BF16, tag="M")
                        nc.vector.scalar_tensor_tensor(
                            M[:],
                            est8[:, h, :],
                            dsp[:, cc, h : h + 1],
                            bcm[:],
                            ALU.mult,
                            ALU.mult,
                        )
                        nc.tensor.matmul(
                            py[:, hsl], M[:], xT[:, hsl], start=True, stop=False
                        )
                        nc.tensor.matmul(
                            py[:, hsl],
                            did_sb[:, h, :],
                            xT[:, hsl],
                            start=False,
                            stop=first,
                        )
                        if not first:
                            nc.tensor.matmul(
                                py[:, hsl],
                                Ce[:, h, :],
                                state["R"][:, h, :],
                                start=False,
                                stop=True,
                            )

                    y_sb = yp.tile([128, XC], BF16, tag="y", name=f"y{g}")
                    Rn = rp.tile([128, HPC, HD], BF16, tag="R", name=f"R{g}")
                    if first:
                        nc.scalar.copy(Rn[:].rearrange("p h d -> p (h d)"), pst[:])
                    else:
                        for h in range(HPC):
                            hsl = slice(h * HD, (h + 1) * HD)
                            nc.vector.scalar_tensor_tensor(
                                Rn[:, h, :],
                                state["R"][:, h, :],
                                eal[:, 0, h : h + 1],
                                pst[:, hsl],
                                ALU.mult,
                                ALU.add,
                            )
                    nc.scalar.copy(y_sb[:], py[:])
                    state["R"] = Rn

                    if g == 0:
                        state["yg"] = ygp.tile(
                            [128, NCH, XC], BF16, tag="yg", name="yg"
                        )
                    yg = state["yg"]
                    nc.gpsimd.tensor_tensor(
                        yg[:, g, :], y_sb[:], zs[:, cc, :], ALU.mult
                    )
                    sq = hw2.tile([128, XC], BF16, tag="tz")
                    nc.scalar.activation(
                        sq[:],
                        yg[:, g, :],
                        AF.Square,
                        accum_out=partial[:, g : g + 1],
                    )
                    if g == 29:
                        issue_ag(2)      # chunks 24-29

            def issue_ag(gi):
                g0, n = AG_OFF[gi], AG_SIZES[gi]
                nc.gpsimd.dma_start(cc_in[gi][:], partial[:, g0 : g0 + n])
                nc.gpsimd.collective_compute(
                    "AllGather",
                    ALU.bypass,
                    replica_groups=[list(range(NCORE))],
                    ins=[cc_in[gi].opt()],
                    outs=[cc_out[gi].opt()],
                )

            def norm_group(gi):
                g0, n = AG_OFF[gi], AG_SIZES[gi]
                yg = state["yg"]
                ag = otp.tile([128, NCORE, n], F32, tag="ag", name=f"ag{gi}")
                nc.sync.dma_start(ag[:], cc_out[gi][:].rearrange("g p b -> p g b"))
                tot = otp.tile([128, n], F32, tag="tot", name=f"tot{gi}")
                nc.vector.tensor_tensor(tot[:], ag[:, 0, :], ag[:, 1, :], ALU.add)
                for c in range(2, NCORE):
                    nc.vector.tensor_tensor(tot[:], tot[:], ag[:, c, :], ALU.add)
                nc.vector.tensor_scalar(
                    tot[:], tot[:], 1.0 / DI, EPS, ALU.mult, ALU.add
                )
                rec = otp.tile([128, n], F32, tag="rec", name=f"rc{gi}")
                nc.vector.reciprocal(rec[:], tot[:])
                scl = otp.tile([128, n], F32, tag="scl", name=f"sc{gi}")
                nc.scalar.activation(scl[:], rec[:], AF.Sqrt)
                for j in range(n):
                    g = g0 + j
                    nc.vector.tensor_scalar(
                        yg[:, g, :], yg[:, g, :], scl[:, j : j + 1], None,
                        ALU.mult, ALU.bypass,
                    )
                    nc.vector.tensor_tensor(
                        yg[:, g, :], yg[:, g, :], nrm_sb[:], ALU.mult
                    )
                # one DMA for the whole group: out rows are (g, s) while yg is
                # [s, g, :]
                nc.sync.dma_start(
                    out.rearrange("(a s) c -> s a c", s=128)[:, g0 : g0 + n, :],
                    yg[:, g0 : g0 + n, :],
                )

            # ---------------- pipelined main loop -------------------------
            dt_chain(0)
            issue_pab(0)
            issue_pab(1)
            for p in range(NBLK):
                if p + 1 < NBLK:
                    prefetch_ut(p + 1)
                if p >= 1:
                    issue_transposes(p - 1)
                in_proj_block(p)
                if p >= 1:
                    ssd_block(p - 1)
                # collectives: issue one iteration after the partials are
                # complete so the input-DMA wait never blocks a queue
                if p == 5:
                    issue_ag(0)          # chunks 0-15 (ready end of iter 4)
                if p == 7:
                    norm_group(0)
                    issue_ag(1)          # chunks 16-23 (ready end of iter 6)

            issue_transposes(NBLK - 1)
            ssd_block(NBLK - 1)
            issue_ag(3)                  # chunks 30-31
            norm_group(1)
            norm_group(2)
            norm_group(3)

    nc.compile()
    return nc


_NC = None


def _host_inputs(u, w_in, conv_w, conv_b, dt_bias, A_log, D_skip, norm_w):
    u2 = np.ascontiguousarray(u.reshape(T, DM).T).astype(BF)
    tri = np.tril(np.ones((128, 128), np.float32)).T.copy()  # tri[i,s]=1 if i<=s
    mask = np.zeros((128, CHK), np.float32)
    for s in range(128):
        mask[s, s:] = 1.0                                    # keep t >= s
    idf = np.eye(128, dtype=np.float32)

    ins = []
    for k in range(NCORE):
        xcols = np.arange(DI + k * XC, DI + (k + 1) * XC)
        bcols = np.arange(2 * DI, 2 * DI + 2 * DS)
        dtcols = np.arange(2 * DI + 2 * DS + k * HPC, 2 * DI + 2 * DS + (k + 1) * HPC)
        wxbc = np.ascontiguousarray(
            np.concatenate([w_in[:, xcols], w_in[:, bcols]], 1)
        ).astype(BF)
        wzk = np.ascontiguousarray(0.5 * w_in[:, k * XC : (k + 1) * XC]).astype(BF)
        wdtk = np.ascontiguousarray(w_in[:, dtcols]).astype(BF)
        chans = np.concatenate(
            [np.arange(k * XC, (k + 1) * XC), np.arange(DI, DI + 2 * DS)]
        )
        cw = 0.5 * conv_w[chans]                             # [768, 4]
        cbv = conv_b[chans]                                  # [768]
        sw = conv_w[chans].sum(1)
        cshift = np.where(np.abs(sw) > 1e-6, cbv / np.where(sw == 0, 1, sw), 0.0)
        cwd = np.zeros((128, 24, 128), np.float32)
        cv = np.zeros((128, 6), np.float32)
        hv = np.zeros((128, 6, 3), np.float32)
        for cbk in range(6):
            blk_w = cw[cbk * 128 : (cbk + 1) * 128]          # [128, 4]
            for kk in range(4):
                cwd[:, 4 * cbk + kk, :] = np.diag(blk_w[:, kk])
            cv[:, cbk] = cshift[cbk * 128 : (cbk + 1) * 128]
            hv[:, cbk, :] = cshift[cbk * 128 : (cbk + 1) * 128, None]
        dtb = np.tile(dt_bias[None, k * HPC : (k + 1) * HPC], (128, 4)).astype(
            np.float32
        )
        ab = np.tile(
            -np.exp(A_log[None, k * HPC : (k + 1) * HPC].astype(np.float64)).astype(
                np.float32
            ),
            (128, 4),
        )
        did = np.zeros((128, HPC * 128), np.float32)
        for h in range(HPC):
            did[:, h * 128 : (h + 1) * 128] = (
                np.eye(128, dtype=np.float32) * D_skip[k * HPC + h]
            )
        nrm = np.tile(norm_w[None, k * XC : (k + 1) * XC], (128, 1))
        ins.append(
            dict(
                uT=u2,
                wxbc=wxbc,
                wz=wzk,
                wdt=wdtk,
                convd=np.ascontiguousarray(
                    cwd.reshape(128, 24 * 128).astype(BF)
                ),
                cvec=np.ascontiguousarray(cv),
                hvec=np.ascontiguousarray(hv.reshape(128, 18)),
                dtbb=np.ascontiguousarray(dtb),
                abc=np.ascontiguousarray(ab),
                dids=np.ascontiguousarray(did.astype(BF)),
                nrmb=np.ascontiguousarray(nrm.astype(BF)),
                trim=np.ascontiguousarray(tri),
                maskm=mask,
                idf=idf,
            )
        )
    return ins


def kernel(u, w_in, conv_w, conv_b, dt_bias, A_log, D_skip, norm_w):
    global _NC
    u = np.asarray(u, np.float32)
    w_in = np.asarray(w_in, np.float32)
    conv_w = np.asarray(conv_w, np.float32)
    conv_b = np.asarray(conv_b, np.float32)
    dt_bias = np.asarray(dt_bias, np.float32)
    A_log = np.asarray(A_log, np.float32)
    D_skip = np.asarray(D_skip, np.float32)
    norm_w = np.asarray(norm_w, np.float32)

    if _NC is None:
        _NC = _build_nc()
    ins = _host_inputs(u, w_in, conv_w, conv_b, dt_bias, A_log, D_skip, norm_w)
    res = run_bass_kernel_spmd(_NC, ins, core_ids=list(range(NCORE)))
    full = np.concatenate(
        [np.asarray(res.results[k]["out"], np.float32) for k in range(NCORE)], axis=1
    )
    return full.reshape(B_, L_, DI)
